# revision 11
# baseline (speedup 1.0000x reference)
"""LocalAttention (B=1, S=4096, D=1024, H=16, hd=64, window=128) on 8 trn2 cores.

Sharding: sequence-parallel. Core c owns queries [512c, 512c+512) and receives
a key/value halo slice of 768 rows ([512c-128, 512c+640), zero-padded at the
global edges). All projection weights are replicated (bf16); device compute is
bf16 with fp32 PSUM accumulation.

v2 layout/schedule (vs v1 baseline):
  - Warmup matmul stream at t=0 (memset tile) so the PE HAM clock-gate opens
    during the input DMA instead of during the first projection.
  - Inputs stream in db-chunks (128 contraction rows) ordered K,Q,V,Wo and the
    K/Q/V projections run db-outer in psum-bank passes, so the first GEMM
    starts ~2us in and DMA stays hidden behind compute.
  - Attention is qb-outer, h-inner, software-pipelined: scores/exp/mask run
    `LAG` heads ahead of PV so the PE never head-of-line blocks on the
    scalar-engine exp or the DVE mask of the same head.
  - PV accumulates 4 heads per psum bank ([128, 4*65]); softmax denominators
    (ones-column, col 64 of each head slab) are gathered per group, one
    reciprocal + one broadcast multiply per group normalizes and writes ao.
  - Per qb: 8 PE transposes (ao -> aot) and a 16-matmul output projection,
    overlapped with the next qb's scores via psum pool rotation.
"""

import os

import numpy as np
import ml_dtypes

import concourse.bass as bass
import concourse.bacc as bacc
import concourse.mybir as mybir
import concourse.tile as tile
from concourse.bass_utils import run_bass_kernel_spmd

BF16 = mybir.dt.bfloat16
FP32 = mybir.dt.float32

NCORES = 8
S = 4096
D = 1024
H = 16
HD = 64
E = H * HD  # 1024
WIN = 128
SL = S // NCORES       # 512 queries per core
SK = SL + 2 * WIN      # 768 keys/values incl. halo
NQB = SL // 128        # 4 query blocks
NKB = SK // 128        # 6 key blocks
NDB = D // 128         # 8 contraction blocks
NEB = E // 128         # 8 embed blocks
VROW = HD + 1          # 65: v columns per head incl. ones column
LAG = 4                # heads of scores/exp/mask lead over PV

_CACHE = {}
LAST_RESULT = None  # BassKernelResults of the most recent run (for test.py)


def _build_nc():
    nc = bacc.Bacc("TRN2", target_bir_lowering=False, debug=False)

    qt_d = nc.dram_tensor("qt", [D, SL], BF16, kind="ExternalInput").ap()
    kt_d = nc.dram_tensor("kt", [D, SK], BF16, kind="ExternalInput").ap()
    vt_d = nc.dram_tensor("vt", [D, SK], BF16, kind="ExternalInput").ap()
    wq_d = nc.dram_tensor("wq", [D, E], BF16, kind="ExternalInput").ap()
    wk_d = nc.dram_tensor("wk", [D, E], BF16, kind="ExternalInput").ap()
    wv_d = nc.dram_tensor("wv", [D, E], BF16, kind="ExternalInput").ap()
    wo_d = nc.dram_tensor("wo", [E, D], BF16, kind="ExternalInput").ap()
    msk_d = nc.dram_tensor("msk", [NQB * 3, 128, 128], BF16, kind="ExternalInput").ap()
    idn_d = nc.dram_tensor("idn", [128, 128], BF16, kind="ExternalInput").ap()
    out_d = nc.dram_tensor("out", [SL, D], FP32, kind="ExternalOutput").ap()

    with tile.TileContext(nc) as tc:
        pools = []

        def pool(name, bufs, **kw):
            p = tc.tile_pool(name=name, bufs=bufs, **kw)
            pools.append(p)
            return p.__enter__()

        const = pool("const", 1)
        pmain = pool("pmain", 2, space="PSUM")
        ep = pool("expp", 8)
        osbp = pool("osb", 2)
        rdp = pool("rd", 2)

        # ---- persistent SBUF tensors ----
        wq_sb = const.tile([128, NDB * E], BF16, tag="wq")
        wk_sb = const.tile([128, NDB * E], BF16, tag="wk")
        wv_sb = const.tile([128, NDB * E], BF16, tag="wv")
        wo_sb = const.tile([128, NEB * D], BF16, tag="wo")
        qtin_sb = const.tile([128, NDB * SL], BF16, tag="qtin")
        ktin_sb = const.tile([128, NDB * SK], BF16, tag="ktin")
        vtin_sb = const.tile([128, NDB * SK], BF16, tag="vtin")
        qt_sb = const.tile([128, NEB * SL], BF16, tag="qt")    # [e,s] per e-blk
        kt_sb = const.tile([128, NEB * SK], BF16, tag="kt")
        v_sb = const.tile([128, NKB * H * VROW], BF16, tag="v")  # [s, h*65]
        msk_sb = const.tile([128, NQB * 3 * 128], BF16, tag="msk")
        idn_sb = const.tile([128, 128], BF16, tag="idn")
        ao_sb = const.tile([128, NQB * E], BF16, tag="ao")     # attn out [sq, e]
        aot_sb = const.tile([128, NEB * SL], BF16, tag="aot")  # transposed [e, sq]
        wrm_sb = const.tile([128, 128], BF16, tag="wrm")

        sync = nc.sync

        # ---- warmup: open the HAM clock gate while DMA streams in ----
        nc.vector.memset(wrm_sb[:], 0.0)
        for i in range(24):
            pw = pmain.tile([128, 512], FP32, tag="pout", bufs=2, name="pw")
            nc.tensor.matmul(pw[:, :128], lhsT=wrm_sb[:], rhs=wrm_sb[:],
                             start=True, stop=True)

        # ---- input DMAs: db-chunked, in consumption order K, Q, V, Wo ----
        for db in range(NDB):
            sync.dma_start(ktin_sb[:, db * SK:(db + 1) * SK],
                           kt_d[db * 128:(db + 1) * 128, :])
            sync.dma_start(wk_sb[:, db * E:(db + 1) * E],
                           wk_d[db * 128:(db + 1) * 128, :])
        for db in range(NDB):
            sync.dma_start(qtin_sb[:, db * SL:(db + 1) * SL],
                           qt_d[db * 128:(db + 1) * 128, :])
            sync.dma_start(wq_sb[:, db * E:(db + 1) * E],
                           wq_d[db * 128:(db + 1) * 128, :])
        for db in range(NDB):
            sync.dma_start(vtin_sb[:, db * SK:(db + 1) * SK],
                           vt_d[db * 128:(db + 1) * 128, :])
            sync.dma_start(wv_sb[:, db * E:(db + 1) * E],
                           wv_d[db * 128:(db + 1) * 128, :])
        for eb in range(NEB):
            sync.dma_start(wo_sb[:, eb * D:(eb + 1) * D],
                           wo_d[eb * 128:(eb + 1) * 128, :])
        sync.dma_start(
            msk_sb[:].rearrange("p (m c) -> p m c", c=128),
            msk_d.rearrange("m p c -> p m c"),
        )
        sync.dma_start(idn_sb[:], idn_d[:])

        # ones columns of v_sb (col hd=64 of each head group)
        v3 = v_sb[:].rearrange("p (k h c) -> p k h c", k=NKB, h=H)
        nc.gpsimd.memset(v3[:, :, :, HD:VROW], 1.0)

        # ---- K projection: [e,s] = Wk[d,e].T @ KT[d,s]; db-outer passes ----
        for p4 in range(4):
            ebs = (2 * p4, 2 * p4 + 1)
            ps = {}
            for i, eb in enumerate(ebs):
                ps[eb, 0] = pmain.tile([128, 512], FP32, tag="pout", bufs=2, name="pk0")
                ps[eb, 1] = pmain.tile([128, 512], FP32, tag="scr", bufs=4, name="pk1")
            for db in range(NDB):
                for eb in ebs:
                    lw = wk_sb[:, db * E + eb * 128: db * E + (eb + 1) * 128]
                    nc.tensor.matmul(ps[eb, 0][:], lhsT=lw,
                                     rhs=ktin_sb[:, db * SK: db * SK + 512],
                                     start=(db == 0), stop=(db == NDB - 1))
                    nc.tensor.matmul(ps[eb, 1][:, :256], lhsT=lw,
                                     rhs=ktin_sb[:, db * SK + 512: db * SK + SK],
                                     start=(db == 0), stop=(db == NDB - 1))
            for eb in ebs:
                nc.vector.tensor_copy(kt_sb[:, eb * SK: eb * SK + 512],
                                      ps[eb, 0][:])
                nc.vector.tensor_copy(kt_sb[:, eb * SK + 512: eb * SK + SK],
                                      ps[eb, 1][:, :256])

        # ---- Q projection: db-outer passes of 4 e-blocks ----
        for p2 in range(2):
            ebs = tuple(range(4 * p2, 4 * p2 + 4))
            ps = {}
            for i, eb in enumerate(ebs):
                tag = "pout" if i % 2 == 0 else "scr"
                ps[eb] = pmain.tile([128, 512], FP32, tag=tag, name="pq",
                                    bufs=2 if tag == "pout" else 4)
            for db in range(NDB):
                for eb in ebs:
                    nc.tensor.matmul(
                        ps[eb][:],
                        lhsT=wq_sb[:, db * E + eb * 128: db * E + (eb + 1) * 128],
                        rhs=qtin_sb[:, db * SL: db * SL + SL],
                        start=(db == 0), stop=(db == NDB - 1))
            for eb in ebs:
                nc.vector.tensor_copy(qt_sb[:, eb * SL:(eb + 1) * SL], ps[eb][:])

        # ---- V projection (natural): [s,e] = VT[d,s].T @ Wv[d,e] ----
        for p3 in range(3):
            kbs = (2 * p3, 2 * p3 + 1)
            ps = {}
            for i, kb in enumerate(kbs):
                ps[kb, 0] = pmain.tile([128, 512], FP32, tag="pout", bufs=2, name="pv0")
                ps[kb, 1] = pmain.tile([128, 512], FP32, tag="scr", bufs=4, name="pv1")
            for db in range(NDB):
                for kb in kbs:
                    lw = vtin_sb[:, db * SK + kb * 128: db * SK + (kb + 1) * 128]
                    nc.tensor.matmul(ps[kb, 0][:], lhsT=lw,
                                     rhs=wv_sb[:, db * E: db * E + 512],
                                     start=(db == 0), stop=(db == NDB - 1))
                    nc.tensor.matmul(ps[kb, 1][:], lhsT=lw,
                                     rhs=wv_sb[:, db * E + 512: db * E + E],
                                     start=(db == 0), stop=(db == NDB - 1))
            for kb in kbs:
                for eh in range(2):
                    dst = v3[:, kb, eh * 8:(eh + 1) * 8, 0:HD]
                    src = ps[kb, eh][:].rearrange("p (h c) -> p h c", c=HD)
                    nc.scalar.copy(dst, src)

        # ---- attention: qb-outer, h-inner, LAG-head software pipeline ----
        # Per 4-head group g: PV -> normalize (DVE) -> 2 transposes (e-blocks
        # 2g, 2g+1) -> 4 output-projection matmuls accumulate into po[qb].
        # The transpose/outproj chunk is deferred one step so the next head's
        # score matmuls cover the normalize latency on the PE.
        scale = 1.0 / np.sqrt(HD)
        po_map = {}
        pend = [None]

        def chunk(qb, g):
            if g == 0:
                po_map[qb] = [
                    pmain.tile([128, 512], FP32, tag="pout", bufs=2, name="po")
                    for _ in range(2)]
            po = po_map[qb]
            for eb in (2 * g, 2 * g + 1):
                pt = pmain.tile([128, 128], BF16, tag="pv", bufs=2, name="pt")
                nc.tensor.transpose(
                    pt[:], ao_sb[:, qb * E + eb * 128: qb * E + (eb + 1) * 128],
                    idn_sb[:])
                nc.vector.tensor_copy(
                    aot_sb[:, eb * SL + qb * 128: eb * SL + (qb + 1) * 128],
                    pt[:])
            for eb in (2 * g, 2 * g + 1):
                lw = aot_sb[:, eb * SL + qb * 128: eb * SL + (qb + 1) * 128]
                for dh in range(2):
                    nc.tensor.matmul(
                        po[dh][:], lhsT=lw,
                        rhs=wo_sb[:, eb * D + dh * 512: eb * D + (dh + 1) * 512],
                        start=(eb == 0), stop=(eb == NEB - 1))
            if g == 3:
                po = po_map.pop(qb)
                osb = osbp.tile([128, D], FP32, tag="osb")
                for dh in range(2):
                    nc.vector.tensor_copy(osb[:, dh * 512:(dh + 1) * 512],
                                          po[dh][:])
                sync.dma_start(out_d[qb * 128:(qb + 1) * 128, :], osb[:])

        for qb in range(NQB):
            expt = {}
            ug = {}

            def front(h, qb=qb, expt=expt):
                hp = (h % 2) * HD
                he = h // 2
                pscr = pmain.tile([128, 512], FP32, tag="scr", bufs=4)
                for r in range(3):
                    kb = qb + r
                    nc.tensor.matmul(
                        pscr[:, r * 128:(r + 1) * 128],
                        lhsT=kt_sb[hp:hp + HD,
                                   he * SK + kb * 128: he * SK + (kb + 1) * 128],
                        rhs=qt_sb[hp:hp + HD,
                                  he * SL + qb * 128: he * SL + (qb + 1) * 128],
                        start=True, stop=True)
                ex = ep.tile([128, 384], BF16, tag="expp")
                nc.scalar.activation(ex[:], pscr[:, :384],
                                     mybir.ActivationFunctionType.Exp, scale=scale)
                # one strided op masks both diagonal sub-blocks (r=0 and r=2)
                exm = ex[:].rearrange("p (r c) -> p r c", c=128)[:, 0:3:2, :]
                mkm = msk_sb[:].rearrange("p (m c) -> p m c", c=128)[
                    :, qb * 3: qb * 3 + 3: 2, :]
                nc.vector.tensor_mul(exm, exm, mkm)
                expt[h] = ex

            def pv(h, qb=qb, expt=expt, ug=ug):
                g, hi = h // 4, h % 4
                if hi == 0:
                    ug[g] = pmain.tile([128, 4 * VROW], FP32, tag="pv", bufs=2, name="pu")
                ex = expt.pop(h)
                for i, r in enumerate((1, 0, 2)):
                    kb = qb + r
                    nc.tensor.matmul(
                        ug[g][:, hi * VROW:(hi + 1) * VROW],
                        lhsT=ex[:, r * 128:(r + 1) * 128],
                        rhs=v_sb[:, (kb * H + h) * VROW:(kb * H + h + 1) * VROW],
                        start=(i == 0), stop=(i == 2))

            def norm(g, qb=qb, ug=ug):
                u = ug.pop(g)
                u3 = u[:].rearrange("p (h c) -> p h c", c=VROW)
                rd = rdp.tile([128, 4], FP32, tag="rd")
                nc.vector.reciprocal(rd[:].unsqueeze(2), u3[:, :, HD:VROW])
                rb = rd[:].unsqueeze(2).broadcast_to([128, 4, HD])
                dst = ao_sb[:, qb * E + g * 4 * HD: qb * E + (g + 1) * 4 * HD]
                nc.vector.tensor_mul(
                    dst.rearrange("p (h c) -> p h c", c=HD),
                    u3[:, :, 0:HD], rb)

            for h in range(16 + LAG):
                if h < 16:
                    front(h)
                if pend[0] is not None:
                    chunk(*pend[0])
                    pend[0] = None
                if h >= LAG:
                    pv(h - LAG)
                    if (h - LAG) % 4 == 3:
                        norm((h - LAG) // 4)
                        pend[0] = (qb, (h - LAG) // 4)

        if pend[0] is not None:
            chunk(*pend[0])
            pend[0] = None

        for p in reversed(pools):
            p.__exit__(None, None, None)

    nc.compile()
    return nc


def _host_inputs(query, key, value, Wq, Wk, Wv, Wo):
    bf = ml_dtypes.bfloat16
    q2 = np.ascontiguousarray(query.reshape(S, D))
    k2 = np.asarray(key).reshape(S, D)
    v2 = np.asarray(value).reshape(S, D)
    kpad = np.zeros((S + 2 * WIN, D), np.float32)
    kpad[WIN:WIN + S] = k2
    vpad = np.zeros((S + 2 * WIN, D), np.float32)
    vpad[WIN:WIN + S] = v2

    wq = np.ascontiguousarray(Wq.astype(bf))
    wk = np.ascontiguousarray(Wk.astype(bf))
    wv = np.ascontiguousarray(Wv.astype(bf))
    wo = np.ascontiguousarray(Wo.astype(bf))
    idn = np.eye(128, dtype=bf)

    kt = np.arange(128)[:, None]
    qi = np.arange(128)[None, :]
    tri0 = (qi <= kt).astype(bf)
    tri2 = (kt <= qi).astype(bf)
    ones = np.ones((128, 128), bf)
    zeros = np.zeros((128, 128), bf)

    in_maps = []
    for c in range(NCORES):
        s0 = c * SL
        qt = np.ascontiguousarray(q2[s0:s0 + SL].T.astype(bf))
        ktc = np.ascontiguousarray(kpad[s0:s0 + SK].T.astype(bf))
        vtc = np.ascontiguousarray(vpad[s0:s0 + SK].T.astype(bf))
        msk = np.empty((NQB * 3, 128, 128), bf)
        for qb in range(NQB):
            m0 = zeros if (c == 0 and qb == 0) else tri0
            m2 = zeros if (c == NCORES - 1 and qb == NQB - 1) else tri2
            msk[qb * 3 + 0] = m0
            msk[qb * 3 + 1] = ones
            msk[qb * 3 + 2] = m2
        in_maps.append({
            "qt": qt, "kt": ktc, "vt": vtc,
            "wq": wq, "wk": wk, "wv": wv, "wo": wo,
            "msk": msk, "idn": idn,
        })
    return in_maps


def kernel(query, key, value, Wq, Wk, Wv, Wo):
    global LAST_RESULT
    if "nc" not in _CACHE:
        _CACHE["nc"] = _build_nc()
    nc = _CACHE["nc"]
    in_maps = _host_inputs(
        np.asarray(query), np.asarray(key), np.asarray(value),
        np.asarray(Wq), np.asarray(Wk), np.asarray(Wv), np.asarray(Wo),
    )
    trace = os.environ.get("KERNEL_TRACE", "0") == "1"
    try:
        res = run_bass_kernel_spmd(
            nc, in_maps, core_ids=list(range(NCORES)), trace=trace
        )
    except ModuleNotFoundError:
        res = run_bass_kernel_spmd(
            nc, in_maps, core_ids=list(range(NCORES)), trace=False
        )
    LAST_RESULT = res
    out = np.concatenate([res.results[c]["out"] for c in range(NCORES)], axis=0)
    return out.reshape(1, S, D).astype(np.float32)
